# revision 19
# baseline (speedup 1.0000x reference)
"""CausalSelfAttention Trainium2 kernel (B=2, T=2048, C=1024, H=16, HS=64).

Sharding (8 cores): core = 4*b + g. Data parallel over batch b in {0,1},
tensor parallel over head-groups g in {0..3} (4 heads / 256 channels each).
Each core computes its heads' attention and a partial output projection
(contracting its 256 channels of w_proj); the host sums the 4 partials per
batch and adds b_proj.

On-device dataflow is fully transposed so no transposes are needed:
  q^T/k^T: [128, T] head-pair tiles (hs on partitions, even head 0:64 /
           odd head 64:128) from lhsT=w_slice, rhs=x^T
  v:       [T, hs] natural (from lhsT=x^T tile, rhs=w_slice), with a ones
           column per head so the PV matmul also accumulates the softmax
           denominator (row 64 of the accumulator).
  S^T:     [Tk, Tq] = (k @ q^T) blocks; softmax along partitions becomes
           free-axis-independent: exp on ACT, denominator via the ones row.
Causal handling: block (i, j) only computes columns >= 128*(i-4j); only the
128-wide diagonal sub-block is masked (one [128,128] tri multiply).
All matmuls run in fp16 (fp32 PSUM accumulation); exp needs no
max-subtraction (scores are ~N(0,1); fp16/fp32 exp range is ample).
The build is software-pipelined by Tq chunk so ACT (exp) starts while PE is
still on later QKV chunks.
"""

import numpy as np

import concourse.bass as bass
import concourse.bacc as bacc
import concourse.mybir as mybir
import concourse.tile as tile
from concourse import bass_utils

F16 = mybir.dt.float16
F32 = mybir.dt.float32

B, T, C, H = 2, 2048, 1024, 16
HS = C // H            # 64
G = 4                  # heads per core
CH = G * HS            # 256 channels per core
NK = C // 128          # 8 contraction tiles for the projections
NT = T // 128          # 16 sequence tiles
NCHUNK = T // 512      # 4 Tq chunks

LAST_RESULT = None


def _build_nc(repeat=1):
    nc = bacc.Bacc("TRN2", target_bir_lowering=False)

    xT = nc.dram_tensor("xT", [C, T], F16, kind="ExternalInput")        # x[b].T
    wqk = nc.dram_tensor("wqk", [C, 2 * CH], F16, kind="ExternalInput")  # [C, q|k]
    wv = nc.dram_tensor("wv", [C, CH], F16, kind="ExternalInput")
    wp = nc.dram_tensor("wp", [CH, C], F16, kind="ExternalInput")        # w_proj rows
    bqk = nc.dram_tensor("bqk", [128, 4], F32, kind="ExternalInput")  # col m: 128-ch block m of [b_q|b_k]
    bv = nc.dram_tensor("bv", [1, CH], F16, kind="ExternalInput")
    mask = nc.dram_tensor("mask", [128, 128], F16, kind="ExternalInput")
    out = nc.dram_tensor("out", [T, C], F32, kind="ExternalOutput")

    with tile.TileContext(nc) as tc:
        with (
            tc.tile_pool(name="p_xT", bufs=NK) as p_xT,
            tc.tile_pool(name="p_wqk", bufs=NK) as p_wqk,
            tc.tile_pool(name="p_wv", bufs=NK) as p_wv,
            tc.tile_pool(name="p_wp", bufs=2) as p_wp,
            tc.tile_pool(name="p_qk", bufs=4) as p_qk,
            tc.tile_pool(name="p_v", bufs=NT) as p_v,
            tc.tile_pool(name="p_pt", bufs=6) as p_pt,
            tc.tile_pool(name="p_yT", bufs=2) as p_yT,
            tc.tile_pool(name="p_out", bufs=4) as p_out,
            tc.tile_pool(name="p_const", bufs=1) as p_const,
            tc.tile_pool(name="p_rc", bufs=4) as p_rc,
            tc.tile_pool(name="p_bc", bufs=4) as p_bc,
            tc.tile_pool(name="ps_mm", bufs=4, space="PSUM") as ps_mm,
            tc.tile_pool(name="ps_y", bufs=4, space="PSUM") as ps_y,
        ):
            # ---- loads ----
            xT_sb = []
            for k in range(NK):
                t = p_xT.tile([128, T], F16, tag="xT")
                nc.sync.dma_start(out=t, in_=xT[k * 128:(k + 1) * 128, :])
                xT_sb.append(t)
            wqk_sb = []
            for k in range(NK):
                t = p_wqk.tile([128, 2 * CH], F16, tag="wqk")
                nc.scalar.dma_start(out=t, in_=wqk[k * 128:(k + 1) * 128, :])
                wqk_sb.append(t)
            wv_sb = []
            for k in range(NK):
                t = p_wv.tile([128, CH], F16, tag="wv")
                nc.gpsimd.dma_start(out=t, in_=wv[k * 128:(k + 1) * 128, :])
                wv_sb.append(t)
            wp_sb = []
            for c in range(2):
                t = p_wp.tile([128, C], F16, tag="wp")
                nc.gpsimd.dma_start(out=t, in_=wp[c * 128:(c + 1) * 128, :])
                wp_sb.append(t)
            bqk_sb = p_const.tile([128, 4], F32, tag="bqk")
            nc.gpsimd.dma_start(out=bqk_sb, in_=bqk[:, :])
            bv_sb = p_const.tile([1, CH], F16, tag="bv")
            nc.gpsimd.dma_start(out=bv_sb, in_=bv[:, :])
            tri = p_const.tile([128, 128], F16, tag="tri")  # 1 iff k <= q
            nc.gpsimd.dma_start(out=tri, in_=mask[:, :])
            ones_sb = p_const.tile([1, 512], F16, tag="ones")
            nc.vector.memset(ones_sb, 1.0)

            # q/k head-pair tiles [128, T] fp16: partitions 0:64 = even head,
            # 64:128 = odd head.  0,1 = q pairs; 2,3 = k pairs.
            qk_sb = [
                p_qk.tile([128, T], F16, tag="qk", name=f"qk{i}") for i in range(4)
            ]
            v_sb = [
                p_v.tile([128, G, HS + 1], F16, tag="v", name=f"v{i}")
                for i in range(NT)
            ]
            yT_sb = [
                p_yT.tile([128, T], F16, tag="yT", name=f"yT{i}") for i in range(2)
            ]

            def qk_chunk(m, j):
                """q (m=0,1) / k (m=2,3) projection for Tq chunk j."""
                ps = ps_mm.tile([128, 512], F32, tag="mm", name="ps_qk")
                for k in range(NK):
                    nc.tensor.matmul(
                        ps,
                        lhsT=wqk_sb[k][:, m * 128:(m + 1) * 128],
                        rhs=xT_sb[k][:, j * 512:(j + 1) * 512],
                        start=(k == 0),
                        stop=(k == NK - 1),
                    )
                nc.vector.tensor_scalar_add(
                    out=qk_sb[m][:, j * 512:(j + 1) * 512],
                    in0=ps,
                    scalar1=bqk_sb[:, m:m + 1],
                )

            def v_tile(t):
                """v projection for Tk tile t (+bias, + ones column)."""
                ps = ps_mm.tile([128, 512], F32, tag="mm", name="ps_v")
                for k in range(NK):
                    nc.tensor.matmul(
                        ps[:, 0:CH],
                        lhsT=xT_sb[k][:, t * 128:(t + 1) * 128],
                        rhs=wv_sb[k],
                        start=(k == 0),
                        stop=False,
                    )
                nc.tensor.matmul(
                    ps[:, 0:CH],
                    lhsT=ones_sb[:, 0:128],
                    rhs=bv_sb,
                    start=False,
                    stop=True,
                )
                vt = v_sb[t]
                nc.vector.tensor_copy(
                    out=vt[:, :, 0:HS],
                    in_=ps[:, 0:CH].rearrange("p (g d) -> p g d", g=G),
                )
                nc.vector.memset(vt[:, :, HS:HS + 1], 1.0)

            def attn_chunk(c, j):
                """Attention for head pair c (heads 2c, 2c+1), Tq chunk j."""
                qp = qk_sb[c]
                kp = qk_sb[2 + c]
                nblk = 4 * j + 4
                psy = [
                    ps_y.tile([HS + 1, 512], F32, tag="y", name=f"psy{s}")
                    for s in range(2)
                ]
                for i in range(nblk):
                    r = i - 4 * j
                    c0 = max(0, r) * 128  # first causally-valid column
                    pts = []
                    for s in range(2):  # sub-head at partitions 64s:64s+64
                        p0 = HS * s
                        pss = ps_mm.tile([128, 512], F32, tag="mm", name="ps_s")
                        nc.tensor.matmul(
                            pss[:, c0:512],
                            lhsT=kp[p0:p0 + HS, i * 128:(i + 1) * 128],
                            rhs=qp[p0:p0 + HS, j * 512 + c0:(j + 1) * 512],
                            start=True,
                            stop=True,
                        )
                        pt = p_pt.tile([128, 512], F16, tag="pt")
                        nc.scalar.activation(
                            out=pt[:, c0:512],
                            in_=pss[:, c0:512],
                            func=mybir.ActivationFunctionType.Exp,
                            scale=float(1.0 / np.sqrt(HS)),
                        )
                        if r >= 0:  # mask the 128-wide diagonal sub-block
                            nc.vector.tensor_mul(
                                out=pt[:, c0:c0 + 128],
                                in0=pt[:, c0:c0 + 128],
                                in1=tri,
                            )
                        pts.append(pt)
                    for s in range(2):
                        nc.tensor.matmul(
                            psy[s][:, c0:512],
                            lhsT=v_sb[i][:, 2 * c + s, :],
                            rhs=pts[s][:, c0:512],
                            start=(i == 0),
                            stop=(i == nblk - 1),
                        )
                for s in range(2):
                    rc = p_rc.tile([1, 512], F16, tag="rc")
                    with nc.allow_low_precision(reason="softmax 1/denom fp16"):
                        nc.vector.reciprocal(out=rc, in_=psy[s][HS:HS + 1, :])
                    psb = ps_mm.tile([128, 512], F32, tag="mm", name="ps_b")[0:HS, :]
                    nc.tensor.matmul(
                        psb, lhsT=ones_sb[:, 0:HS], rhs=rc, start=True, stop=True
                    )
                    bc = p_bc.tile([HS, 512], F16, tag="bc")
                    nc.vector.tensor_copy(out=bc, in_=psb)
                    nc.vector.tensor_mul(
                        out=yT_sb[c][HS * s:HS * s + HS, j * 512:(j + 1) * 512],
                        in0=psy[s][0:HS, :],
                        in1=bc,
                    )

            def proj_tile(t):
                """Output projection for Tq tile t."""
                osb = p_out.tile([128, C], F32, tag="os")
                for o in range(2):
                    ps = ps_mm.tile([128, 512], F32, tag="mm", name="ps_o")
                    for c in range(2):
                        nc.tensor.matmul(
                            ps,
                            lhsT=yT_sb[c][:, t * 128:(t + 1) * 128],
                            rhs=wp_sb[c][:, o * 512:(o + 1) * 512],
                            start=(c == 0),
                            stop=(c == 1),
                        )
                    nc.vector.tensor_copy(out=osb[:, o * 512:(o + 1) * 512], in_=ps)
                nc.sync.dma_start(out=out[t * 128:(t + 1) * 128, :], in_=osb)

            # ---- software pipeline over Tq chunks ----
            # chunk j phase: q/k for chunk j (all pairs), v tiles 4j..4j+3,
            # then attention for chunk j (needs v tiles 0..4j+3, in-phase),
            # then the output projection for the chunk's Tq tiles.
            def body():
                for j in range(NCHUNK):
                    for m in range(4):
                        qk_chunk(m, j)
                    for t in range(4 * j, 4 * j + 4):
                        v_tile(t)
                    for c in range(2):
                        attn_chunk(c, j)
                    for t in range(4 * j, 4 * j + 4):
                        proj_tile(t)

            if repeat == 1:
                body()
            else:  # benchmarking only: loop the whole compute on-device
                with tc.For_i(0, repeat, 1):
                    body()

    nc.finalize()
    return nc


_NC = None


def _get_nc():
    global _NC
    if _NC is None:
        _NC = _build_nc()
    return _NC


def _make_in_maps(x, w_attn, b_attn, w_proj):
    mask = (np.arange(128)[:, None] <= np.arange(128)[None, :]).astype(np.float16)
    in_maps = []
    for core in range(8):
        b, g = divmod(core, 4)
        c0 = CH * g
        xTb = np.ascontiguousarray(x[b].T).astype(np.float16)
        wqk = np.concatenate(
            [w_attn[:, c0:c0 + CH], w_attn[:, C + c0:C + c0 + CH]], axis=1
        ).astype(np.float16)
        wv = np.ascontiguousarray(w_attn[:, 2 * C + c0:2 * C + c0 + CH]).astype(
            np.float16
        )
        wp = np.ascontiguousarray(w_proj[c0:c0 + CH, :]).astype(np.float16)
        bqk = np.concatenate(
            [b_attn[c0:c0 + CH], b_attn[C + c0:C + c0 + CH]]
        ).reshape(4, 128).T.astype(np.float32)
        bqk = np.ascontiguousarray(bqk)
        bv = np.ascontiguousarray(
            b_attn[2 * C + c0:2 * C + c0 + CH].reshape(1, CH)
        ).astype(np.float16)
        in_maps.append(
            {
                "xT": xTb,
                "wqk": wqk,
                "wv": wv,
                "wp": wp,
                "bqk": bqk,
                "bv": bv,
                "mask": mask,
            }
        )
    return in_maps


def kernel(x, w_attn, b_attn, w_proj, b_proj, trace=False):
    global LAST_RESULT
    x = np.asarray(x, dtype=np.float32)
    w_attn = np.asarray(w_attn, dtype=np.float32)
    b_attn = np.asarray(b_attn, dtype=np.float32)
    w_proj = np.asarray(w_proj, dtype=np.float32)
    b_proj = np.asarray(b_proj, dtype=np.float32)

    nc = _get_nc()
    in_maps = _make_in_maps(x, w_attn, b_attn, w_proj)
    res = bass_utils.run_bass_kernel_spmd(
        nc, in_maps, core_ids=list(range(8)), trace=trace
    )
    LAST_RESULT = res
    parts = [r["out"] for r in res.results]
    out = np.empty((B, T, C), dtype=np.float32)
    for b in range(B):
        acc = parts[4 * b].astype(np.float32)
        for g in range(1, 4):
            acc = acc + parts[4 * b + g]
        out[b] = acc + b_proj[None, :]
    return out


# revision 20
# speedup vs baseline: 1.0443x; 1.0443x over previous
"""CausalSelfAttention Trainium2 kernel (B=2, T=2048, C=1024, H=16, HS=64).

Sharding (8 cores): core = 4*b + g. Data parallel over batch b in {0,1},
tensor parallel over head-groups g in {0..3} (4 heads / 256 channels each).
Each core computes its heads' attention and a partial output projection
(contracting its 256 channels of w_proj); the host sums the 4 partials per
batch and adds b_proj.

On-device dataflow is fully transposed so no transposes are needed:
  q^T/k^T: [128, T] head-pair tiles (hs on partitions, even head 0:64 /
           odd head 64:128) from lhsT=w_slice, rhs=x^T
  v:       [T, hs] natural (from lhsT=x^T tile, rhs=w_slice), with a ones
           column per head so the PV matmul also accumulates the softmax
           denominator (row 64 of the accumulator).
  S^T:     [Tk, Tq] = (k @ q^T) blocks; softmax along partitions becomes
           free-axis-independent: exp on ACT, denominator via the ones row.
Causal handling: block (i, j) only computes columns >= 128*(i-4j); only the
128-wide diagonal sub-block is masked (one [128,128] tri multiply).
All matmuls run in fp16 (fp32 PSUM accumulation); exp needs no
max-subtraction (scores are ~N(0,1); fp16/fp32 exp range is ample).
The build is software-pipelined by Tq chunk so ACT (exp) starts while PE is
still on later QKV chunks.
"""

import numpy as np

import concourse.bass as bass
import concourse.bacc as bacc
import concourse.mybir as mybir
import concourse.tile as tile
from concourse import bass_utils

F16 = mybir.dt.float16
F32 = mybir.dt.float32

B, T, C, H = 2, 2048, 1024, 16
HS = C // H            # 64
G = 4                  # heads per core
CH = G * HS            # 256 channels per core
NK = C // 128          # 8 contraction tiles for the projections
NT = T // 128          # 16 sequence tiles
NCHUNK = T // 512      # 4 Tq chunks

LAST_RESULT = None


def _build_nc(repeat=1, phases=("qkv", "attn", "proj")):
    nc = bacc.Bacc("TRN2", target_bir_lowering=False)

    xT = nc.dram_tensor("xT", [C, T], F16, kind="ExternalInput")        # x[b].T
    wqk = nc.dram_tensor("wqk", [C, 2 * CH], F16, kind="ExternalInput")  # [C, q|k]
    wv = nc.dram_tensor("wv", [C, CH], F16, kind="ExternalInput")
    wp = nc.dram_tensor("wp", [CH, C], F16, kind="ExternalInput")        # w_proj rows
    bqk = nc.dram_tensor("bqk", [128, 4], F32, kind="ExternalInput")  # col m: 128-ch block m of [b_q|b_k]
    bv = nc.dram_tensor("bv", [1, CH], F16, kind="ExternalInput")
    mask = nc.dram_tensor("mask", [128, 128], F16, kind="ExternalInput")
    out = nc.dram_tensor("out", [T, C], F32, kind="ExternalOutput")

    with tile.TileContext(nc) as tc:
        with (
            tc.tile_pool(name="p_xT", bufs=NK) as p_xT,
            tc.tile_pool(name="p_wqk", bufs=NK) as p_wqk,
            tc.tile_pool(name="p_wv", bufs=NK) as p_wv,
            tc.tile_pool(name="p_wp", bufs=2) as p_wp,
            tc.tile_pool(name="p_qk", bufs=4) as p_qk,
            tc.tile_pool(name="p_v", bufs=NT) as p_v,
            tc.tile_pool(name="p_pt", bufs=6) as p_pt,
            tc.tile_pool(name="p_yT", bufs=2) as p_yT,
            tc.tile_pool(name="p_out", bufs=4) as p_out,
            tc.tile_pool(name="p_const", bufs=1) as p_const,
            tc.tile_pool(name="p_rc", bufs=4) as p_rc,
            tc.tile_pool(name="p_bc", bufs=4) as p_bc,
            tc.tile_pool(name="ps_mm", bufs=4, space="PSUM") as ps_mm,
            tc.tile_pool(name="ps_y", bufs=4, space="PSUM") as ps_y,
        ):
            # ---- loads ----
            xT_sb = []
            for k in range(NK):
                t = p_xT.tile([128, T], F16, tag="xT")
                nc.sync.dma_start(out=t, in_=xT[k * 128:(k + 1) * 128, :])
                xT_sb.append(t)
            wqk_sb = []
            for k in range(NK):
                t = p_wqk.tile([128, 2 * CH], F16, tag="wqk")
                nc.scalar.dma_start(out=t, in_=wqk[k * 128:(k + 1) * 128, :])
                wqk_sb.append(t)
            wv_sb = []
            for k in range(NK):
                t = p_wv.tile([128, CH], F16, tag="wv")
                nc.gpsimd.dma_start(out=t, in_=wv[k * 128:(k + 1) * 128, :])
                wv_sb.append(t)
            wp_sb = []
            for c in range(2):
                t = p_wp.tile([128, C], F16, tag="wp")
                nc.gpsimd.dma_start(out=t, in_=wp[c * 128:(c + 1) * 128, :])
                wp_sb.append(t)
            bqk_sb = p_const.tile([128, 4], F32, tag="bqk")
            nc.gpsimd.dma_start(out=bqk_sb, in_=bqk[:, :])
            bv_sb = p_const.tile([1, CH], F16, tag="bv")
            nc.gpsimd.dma_start(out=bv_sb, in_=bv[:, :])
            tri = p_const.tile([128, 128], F16, tag="tri")  # 1 iff k <= q
            nc.gpsimd.dma_start(out=tri, in_=mask[:, :])
            ones_sb = p_const.tile([1, 512], F16, tag="ones")
            nc.vector.memset(ones_sb, 1.0)

            # q/k head-pair tiles [128, T] fp16: partitions 0:64 = even head,
            # 64:128 = odd head.  0,1 = q pairs; 2,3 = k pairs.
            qk_sb = [
                p_qk.tile([128, T], F16, tag="qk", name=f"qk{i}") for i in range(4)
            ]
            v_sb = [
                p_v.tile([128, G, HS + 1], F16, tag="v", name=f"v{i}")
                for i in range(NT)
            ]
            yT_sb = [
                p_yT.tile([128, T], F16, tag="yT", name=f"yT{i}") for i in range(2)
            ]

            def qk_chunk(m, j):
                """q (m=0,1) / k (m=2,3) projection for Tq chunk j."""
                ps = ps_mm.tile([128, 512], F32, tag="mm", name="ps_qk")
                for k in range(NK):
                    nc.tensor.matmul(
                        ps,
                        lhsT=wqk_sb[k][:, m * 128:(m + 1) * 128],
                        rhs=xT_sb[k][:, j * 512:(j + 1) * 512],
                        start=(k == 0),
                        stop=(k == NK - 1),
                    )
                nc.vector.tensor_scalar_add(
                    out=qk_sb[m][:, j * 512:(j + 1) * 512],
                    in0=ps,
                    scalar1=bqk_sb[:, m:m + 1],
                )

            def v_tile(t):
                """v projection for Tk tile t (+bias, + ones column)."""
                ps = ps_mm.tile([128, 512], F32, tag="mm", name="ps_v")
                for k in range(NK):
                    nc.tensor.matmul(
                        ps[:, 0:CH],
                        lhsT=xT_sb[k][:, t * 128:(t + 1) * 128],
                        rhs=wv_sb[k],
                        start=(k == 0),
                        stop=False,
                    )
                nc.tensor.matmul(
                    ps[:, 0:CH],
                    lhsT=ones_sb[:, 0:128],
                    rhs=bv_sb,
                    start=False,
                    stop=True,
                )
                vt = v_sb[t]
                nc.vector.tensor_copy(
                    out=vt[:, :, 0:HS],
                    in_=ps[:, 0:CH].rearrange("p (g d) -> p g d", g=G),
                )
                nc.vector.memset(vt[:, :, HS:HS + 1], 1.0)

            def attn_chunk(c, j):
                """Attention for head pair c (heads 2c, 2c+1), Tq chunk j."""
                qp = qk_sb[c]
                kp = qk_sb[2 + c]
                nblk = 4 * j + 4
                psy = [
                    ps_y.tile([HS + 1, 512], F32, tag="y", name=f"psy{s}")
                    for s in range(2)
                ]
                for i in range(nblk):
                    r = i - 4 * j
                    c0 = max(0, r) * 128  # first causally-valid column
                    pts = []
                    for s in range(2):  # sub-head at partitions 64s:64s+64
                        p0 = HS * s
                        pss = ps_mm.tile([128, 512], F32, tag="mm", name="ps_s")
                        nc.tensor.matmul(
                            pss[:, c0:512],
                            lhsT=kp[p0:p0 + HS, i * 128:(i + 1) * 128],
                            rhs=qp[p0:p0 + HS, j * 512 + c0:(j + 1) * 512],
                            start=True,
                            stop=True,
                        )
                        pt = p_pt.tile([128, 512], F16, tag="pt")
                        nc.scalar.activation(
                            out=pt[:, c0:512],
                            in_=pss[:, c0:512],
                            func=mybir.ActivationFunctionType.Exp,
                            scale=float(1.0 / np.sqrt(HS)),
                        )
                        if r >= 0:  # mask the 128-wide diagonal sub-block
                            nc.vector.tensor_mul(
                                out=pt[:, c0:c0 + 128],
                                in0=pt[:, c0:c0 + 128],
                                in1=tri,
                            )
                        pts.append(pt)
                    for s in range(2):
                        nc.tensor.matmul(
                            psy[s][:, c0:512],
                            lhsT=v_sb[i][:, 2 * c + s, :],
                            rhs=pts[s][:, c0:512],
                            start=(i == 0),
                            stop=(i == nblk - 1),
                        )
                for s in range(2):
                    rc = p_rc.tile([1, 512], F16, tag="rc")
                    with nc.allow_low_precision(reason="softmax 1/denom fp16"):
                        nc.vector.reciprocal(out=rc, in_=psy[s][HS:HS + 1, :])
                    psb = ps_mm.tile([128, 512], F32, tag="mm", name="ps_b")[0:HS, :]
                    nc.tensor.matmul(
                        psb, lhsT=ones_sb[:, 0:HS], rhs=rc, start=True, stop=True
                    )
                    bc = p_bc.tile([HS, 512], F16, tag="bc")
                    nc.vector.tensor_copy(out=bc, in_=psb)
                    nc.vector.tensor_mul(
                        out=yT_sb[c][HS * s:HS * s + HS, j * 512:(j + 1) * 512],
                        in0=psy[s][0:HS, :],
                        in1=bc,
                    )

            def proj_tile(t):
                """Output projection for Tq tile t."""
                osb = p_out.tile([128, C], F32, tag="os")
                for o in range(2):
                    ps = ps_mm.tile([128, 512], F32, tag="mm", name="ps_o")
                    for c in range(2):
                        nc.tensor.matmul(
                            ps,
                            lhsT=yT_sb[c][:, t * 128:(t + 1) * 128],
                            rhs=wp_sb[c][:, o * 512:(o + 1) * 512],
                            start=(c == 0),
                            stop=(c == 1),
                        )
                    nc.vector.tensor_copy(out=osb[:, o * 512:(o + 1) * 512], in_=ps)
                nc.sync.dma_start(out=out[t * 128:(t + 1) * 128, :], in_=osb)

            # ---- software pipeline over Tq chunks ----
            # chunk j phase: q/k for chunk j (all pairs), v tiles 4j..4j+3,
            # then attention for chunk j (needs v tiles 0..4j+3, in-phase),
            # then the output projection for the chunk's Tq tiles.
            def body():
                for j in range(NCHUNK):
                    if "qkv" in phases:
                        for m in range(4):
                            qk_chunk(m, j)
                        for t in range(4 * j, 4 * j + 4):
                            v_tile(t)
                    if "attn" in phases:
                        for c in range(2):
                            attn_chunk(c, j)
                    if "proj" in phases:
                        for t in range(4 * j, 4 * j + 4):
                            proj_tile(t)

            if repeat == 1:
                body()
            else:  # benchmarking only: loop the whole compute on-device
                with tc.For_i(0, repeat, 1):
                    body()

    nc.finalize()
    return nc


_NC = None


def _get_nc():
    global _NC
    if _NC is None:
        _NC = _build_nc()
    return _NC


def _make_in_maps(x, w_attn, b_attn, w_proj):
    mask = (np.arange(128)[:, None] <= np.arange(128)[None, :]).astype(np.float16)
    in_maps = []
    for core in range(8):
        b, g = divmod(core, 4)
        c0 = CH * g
        xTb = np.ascontiguousarray(x[b].T).astype(np.float16)
        wqk = np.concatenate(
            [w_attn[:, c0:c0 + CH], w_attn[:, C + c0:C + c0 + CH]], axis=1
        ).astype(np.float16)
        wv = np.ascontiguousarray(w_attn[:, 2 * C + c0:2 * C + c0 + CH]).astype(
            np.float16
        )
        wp = np.ascontiguousarray(w_proj[c0:c0 + CH, :]).astype(np.float16)
        bqk = np.concatenate(
            [b_attn[c0:c0 + CH], b_attn[C + c0:C + c0 + CH]]
        ).reshape(4, 128).T.astype(np.float32)
        bqk = np.ascontiguousarray(bqk)
        bv = np.ascontiguousarray(
            b_attn[2 * C + c0:2 * C + c0 + CH].reshape(1, CH)
        ).astype(np.float16)
        in_maps.append(
            {
                "xT": xTb,
                "wqk": wqk,
                "wv": wv,
                "wp": wp,
                "bqk": bqk,
                "bv": bv,
                "mask": mask,
            }
        )
    return in_maps


def kernel(x, w_attn, b_attn, w_proj, b_proj, trace=False):
    global LAST_RESULT
    x = np.asarray(x, dtype=np.float32)
    w_attn = np.asarray(w_attn, dtype=np.float32)
    b_attn = np.asarray(b_attn, dtype=np.float32)
    w_proj = np.asarray(w_proj, dtype=np.float32)
    b_proj = np.asarray(b_proj, dtype=np.float32)

    nc = _get_nc()
    in_maps = _make_in_maps(x, w_attn, b_attn, w_proj)
    res = bass_utils.run_bass_kernel_spmd(
        nc, in_maps, core_ids=list(range(8)), trace=trace
    )
    LAST_RESULT = res
    parts = [r["out"] for r in res.results]
    out = np.empty((B, T, C), dtype=np.float32)
    for b in range(B):
        acc = parts[4 * b].astype(np.float32)
        for g in range(1, 4):
            acc = acc + parts[4 * b + g]
        out[b] = acc + b_proj[None, :]
    return out
